# revision 22
# baseline (speedup 1.0000x reference)
"""Trainium2 Bass kernel for CompanySpecificHeads (MoE-style routed MLP heads).

Semantics (matching the reference):
    out[b] = gelu(z[b] @ W1[cid[b]] + b1[cid[b]]) @ W2[cid[b]] + b2[cid[b]]

Strategy: expert-parallel across 8 NeuronCores. Companies are sharded
8-per-core; tokens are routed (gathered by company) to their company's core
on the host, padded to a fixed per-company capacity, and each core runs a
grouped GEMM -> gelu -> dot pipeline over its 8 companies.

v2 structure (per company c, h on partitions):
  Bias prefill (DVE): the b1 slice for each (c, g) group is broadcast-copied
      into the PSUM bank BEFORE layer-1 runs. Layer-1 matmuls then use
      start=False so the PE accumulates onto the bias. This works because
      the PSUM has_written bits stay set from the previous accumulation
      group on that bank (only start=True clears them); the warmup matmuls
      give every pp bank one full-width PE write before first use so the
      bits are set from the start. Removes the per-group bias selector
      matmul (N=384) from the PE critical path entirely.
  Layer 1 (PE): psum[h, t] += W1[c][d, h] * zT[c][d, t], fp16 moving
      operand, stationary mixed fp16/fp8 (see below), start=False.
  Gelu (ACT): one full-width activation per group, PSUM -> SBUF fp16.
  Layer 2 (PE, deferred): company c's 8 K=128 dot matmuls are emitted
      AFTER company c+1's layer-1, so the gelu of (c, g1) has a full
      company's worth of PE work to hide behind and layer-2 never stalls
      on the ACT engine.

Mixed-precision W1: per (g,k) the first 2 h-chunks (256 cols) are fp16 and
the last 2 h-chunks are fp8e4m3 (PE takes an fp8 stationary with an fp16
moving operand). Host permutes h per company so the smallest-|W2| chunks
take the fp8 slots (gelu is elementwise and layer-2 sums over h, so a
consistent h-permutation of W1/b1/W2 is exact). Packed as raw bytes:
[c][p][g][k][256*2B fp16 | 256*1B fp8], sliced by byte range + bitcast.

DMA: the sync (SP HWDGE) ring carries, in order: wb (w2 + b1, one 48KB
tile), zt[0], w1[0] staggered by k-chunk (compute is k-outer for c0/g0 so
layer-1 starts on the first 0.1MB), zt[1], then w1[1..7] one DMA per
company (completion sems pace the compute). zt[2:] rides the scalar
(ACT HWDGE) ring, dispatched at the head while the ACT engine is idle, so
it never delays the w1 stream. Host does the unshard/scatter back to
[B, 1] and adds b2 (exact, fp32).

PE warmup: the HAM clock gate holds an idle PE at 1.2GHz and takes ~3.4us
of sustained activity to un-throttle to 2.4GHz. A handful of warmup
matmuls on scratch data (memset on the otherwise-idle vector engine)
bridge the framework preamble to the first weight arrival and double as
the has_written coverage for the PSUM banks.
"""

import numpy as np

B, C, D, H = 4096, 64, 512, 1024
NCORES = 8
CPC = C // NCORES  # companies per core
KC = D // 128      # contraction chunks of 128
HC = H // 128      # h chunks of 128

_COMPILED = {}


def _build(TW, NTT, dtype_name):
    """Build the Bass/Tile program for per-company token capacity NTT*TW."""
    import concourse.bass as bass
    import concourse.bacc as bacc
    import concourse.mybir as mybir
    from concourse.tile import TileContext
    from contextlib import ExitStack

    f32 = mybir.dt.float32
    dt_op = getattr(mybir.dt, dtype_name)
    f8 = mybir.dt.float8e4
    u8 = mybir.dt.uint8

    # Packed W1 bytes per (g,k): 2 h-chunks fp16 + 2 h-chunks fp8e4m3.
    W1B = 2 * 128 * 2 + 2 * 128  # = 768 bytes per (g,k)
    # wb layout per partition: [0:2*CPC*HC] w2 fp16, then b1 fp32.
    W2BYTES = CPC * HC * 2          # 128B
    B1BYTES = CPC * 2 * KC * 4      # 256B
    WBW = W2BYTES + B1BYTES

    nc = bacc.Bacc(None, target_bir_lowering=False)

    zt_d = nc.dram_tensor("zt", [128, CPC, NTT, KC, TW], dt_op, kind="ExternalInput")
    w1_d = nc.dram_tensor(
        "w1", [CPC, 128, 2, KC, W1B], u8, kind="ExternalInput"
    )
    wb_d = nc.dram_tensor("wb", [128, WBW], u8, kind="ExternalInput")
    out_d = nc.dram_tensor("out", [1, CPC * NTT * TW], f32, kind="ExternalOutput")

    gelu = mybir.ActivationFunctionType.Gelu

    with TileContext(nc) as tc, ExitStack() as ctx:
        const = ctx.enter_context(tc.tile_pool(name="const", bufs=1))

        # PE warmup scratch: memset on the otherwise-idle vector engine so
        # the warmup matmuls have no dependency on the DMA queues.
        wsc = const.tile([128, KC * TW], dt_op)
        nc.vector.memset(wsc[:], 0.0)

        # The scalar (ACT) HWDGE ring carries ONLY the 48KB wb tile: the
        # scalar ring crawls (~30-100GB/s) whenever the sync stream is
        # active (same 16 SDMA engines, packet round-robin strongly
        # favors the sync queue), so anything compute-gating must ride
        # the sync ring in FIFO need-order.
        wbt = const.tile([128, WBW], u8)
        nc.scalar.dma_start(out=wbt[:], in_=wb_d[:])
        w2t = wbt[:, 0:W2BYTES].bitcast(dt_op)              # [128, CPC*HC]
        b1sb = wbt[:, W2BYTES:WBW].bitcast(f32)             # [128, CPC*2*KC]
        zall = const.tile([128, CPC, NTT, KC, TW], dt_op)
        nc.sync.dma_start(out=zall[:, 0:2], in_=zt_d[:, 0:2])

        # Staged per-company outputs; two sync-ring stores at the end.
        oall = const.tile([1, CPC * NTT * TW], f32)

        # w1 on the sync ring. Transfer sizing balances two measured
        # effects: (a) per-transfer overhead + the 16-engine completion
        # spread reward FEWER, BIGGER transfers (whole-company 786KB
        # pieces sustain ~420GB/s vs ~360 for a finer mix); (b) a company
        # split in g-halves lets its layer-1 g0 start ~1us earlier.
        # So: c0/c1 (head, compute-gating) and c7 (its g1 completion sits
        # on the critical tail) are halved; c2..c6 are whole-company.
        # Token slices zt[2:] are interleaved into the stream ~1 company
        # ahead of need.
        w1p = ctx.enter_context(tc.tile_pool(name="w1p", bufs=1))
        w1ts = []
        for c in range(CPC):
            w1t = w1p.tile([128, 2, KC, W1B], u8, name=f"w1_{c}")
            if c == CPC - 1:
                # tail: g1 in k-halves computed k-outer, so only the last
                # 4 matmuls wait on the stream's final completion sem
                nc.sync.dma_start(out=w1t[:, 0], in_=w1_d[c, :, 0])
                nc.sync.dma_start(out=w1t[:, 1, 0:2], in_=w1_d[c, :, 1, 0:2])
                nc.sync.dma_start(out=w1t[:, 1, 2:4], in_=w1_d[c, :, 1, 2:4])
            else:
                nc.sync.dma_start(out=w1t[:], in_=w1_d[c])
            w1ts.append(w1t)
            if c == 0:
                nc.sync.dma_start(out=zall[:, 2:4], in_=zt_d[:, 2:4])
            elif c == 2:
                nc.sync.dma_start(out=zall[:, 4:6], in_=zt_d[:, 4:6])
            elif c == 4:
                nc.sync.dma_start(out=zall[:, 6:8], in_=zt_d[:, 6:8])

        hp = ctx.enter_context(tc.tile_pool(name="hp", bufs=min(2 * CPC * NTT, 16)))
        pp = ctx.enter_context(tc.tile_pool(name="pp", bufs=5, space="PSUM"))
        opp = ctx.enter_context(tc.tile_pool(name="opp", bufs=2, space="PSUM"))
        wps = ctx.enter_context(tc.tile_pool(name="wps", bufs=1, space="PSUM"))

        # PE warmup matmuls, two jobs: (1) keep the PE's HAM busy-window
        # filling continuously from the end of the framework preamble to
        # the first weight arrival -- any idle gap >= one 4096-cycle
        # window resets the un-throttle progress and the whole stream
        # runs at 1.2GHz instead of 2.4GHz; (2) the first pass through
        # the pp pool gives every pp PSUM bank one full-width PE write so
        # has_written is set before the first bias prefill + start=False
        # accumulation.
        for _ in range(15):
            wp = pp.tile([128, KC * TW], f32, name="ps", tag="ps")
            nc.tensor.matmul(wp[:], wsc[:, :128], wsc[:], start=True, stop=True)

        # Dependency-free fillers on a dedicated bank: bridge the
        # DMA-paced holes at the head of the stream (c0/c1) so the PE
        # stays continuously busy until the weight stream runs ahead.
        wfil = wps.tile([128, 256], f32)

        def filler(n):
            for _ in range(n):
                nc.tensor.matmul(wfil[:], wsc[:, :128], wsc[:, :256],
                                 start=True, stop=True)

        def emit_l2(c, tt, hts):
            osum = opp.tile([1, TW], f32)
            for g in range(2):
                for j in range(KC):
                    jj = KC * g + j
                    nc.tensor.matmul(
                        osum[:],
                        w2t[:, HC * c + jj:HC * c + jj + 1],
                        hts[g][:, j * TW:(j + 1) * TW],
                        start=(jj == 0),
                        stop=(jj == HC - 1),
                    )
            off = (c * NTT + tt) * TW
            nc.vector.tensor_copy(oall[:, off:off + TW], osum[:])

        prev = None
        for c in range(CPC):
            w1t = w1ts[c]
            for tt in range(NTT):
                hts = []
                for g in range(2):
                    ps = pp.tile([128, KC * TW], f32, name="ps", tag="ps")
                    # bias prefill on DVE: ps[128j+m, t] = b1[c][512g+128j+m]
                    bb = b1sb[:, (c * 2 + g) * KC:(c * 2 + g) * KC + KC]
                    nc.vector.tensor_copy(
                        ps.rearrange("p (j t) -> p j t", j=KC),
                        bb.unsqueeze(-1).to_broadcast((128, KC, TW)),
                    )
                    tail = c == CPC - 1 and tt == NTT - 1
                    kj = [(k, j) for k in range(KC) for j in range(KC)] \
                        if (tail and g == 1) else \
                        [(k, j) for j in range(KC) for k in range(KC)]
                    for k, j in kj:
                        if True:
                            if j < 2:
                                lhsT = w1t[:, g, k, 256 * j:256 * (j + 1)].bitcast(
                                    dt_op
                                )
                            else:
                                lhsT = w1t[
                                    :, g, k, 512 + 128 * (j - 2):512 + 128 * (j - 1)
                                ].bitcast(f8)
                            nc.tensor.matmul(
                                ps[:, j * TW:(j + 1) * TW],
                                lhsT,
                                zall[:, c, tt, k, :],
                                start=False,
                                stop=(k == KC - 1),
                                skip_group_check=True,
                            )
                    ht = hp.tile([128, KC * TW], dt_op)
                    if c == CPC - 1 and tt == NTT - 1 and g == 1:
                        # tail: halve the last gelu so layer-2's second
                        # half starts on the first piece
                        half = KC * TW // 2
                        nc.scalar.activation(ht[:, :half], ps[:, :half], gelu)
                        nc.scalar.activation(ht[:, half:], ps[:, half:], gelu)
                    else:
                        nc.scalar.activation(ht[:], ps[:], gelu)
                    hts.append(ht)
                    if c == 0 and tt == 0 and g == 0:
                        filler(6)
                    if tail and g == 0 and prev is not None:
                        # tail: drain the previous company's layer-2
                        # between the last company's two groups, so the
                        # final gelus aren't scheduled behind the
                        # DMA-late L1(g1)
                        emit_l2(*prev)
                        prev = None
                if prev is not None:
                    emit_l2(*prev)
                prev = (c, tt, hts)
            if c == 0:
                filler(6)
            elif c == 1:
                filler(4)
        emit_l2(*prev)

        # Stores on the sync ring: companies 0-6 fire while company 7's
        # tail drains; the final store is tiny.
        osplit = (CPC - 1) * NTT * TW
        nc.sync.dma_start(out=out_d[:, :osplit], in_=oall[:, :osplit])
        nc.sync.dma_start(out=out_d[:, osplit:], in_=oall[:, osplit:])

    nc.finalize()
    return nc


def _get_compiled(TW, NTT, dtype_name):
    key = (TW, NTT, dtype_name)
    if key not in _COMPILED:
        _COMPILED[key] = _build(TW, NTT, dtype_name)
    return _COMPILED[key]


def kernel(z, company_id, W1, b1, W2, b2):
    from concourse.bass_utils import run_bass_kernel_spmd

    z = np.asarray(z, dtype=np.float32)
    cid = np.asarray(company_id).astype(np.int64).ravel()
    W1 = np.asarray(W1, dtype=np.float32)
    b1 = np.asarray(b1, dtype=np.float32)
    W2 = np.asarray(W2, dtype=np.float32)
    b2 = np.asarray(b2, dtype=np.float32)
    O = W2.shape[2]

    np_op = np.float16
    dtype_name = "float16"

    # Per-company h-chunk permutation: the packed layout quantizes chunks
    # 2,3 of each half to fp8, so route the 4 chunks with the SMALLEST
    # |W2| norms into those slots (gelu is elementwise and layer-2 sums
    # over h, so a consistent permutation of W1/b1/W2 along h is exact).
    W1 = W1.copy()
    b1 = b1.copy()
    W2 = W2.copy()
    for gc in range(C):
        norms = (W2[gc, :, 0].reshape(8, 128) ** 2).sum(1)
        order = np.argsort(-norms)
        slot_assign = np.empty(8, dtype=int)
        slot_assign[[0, 1, 4, 5]] = order[:4]
        slot_assign[[2, 3, 6, 7]] = order[4:]
        hidx = (slot_assign[:, None] * 128 + np.arange(128)).ravel()
        W1[gc] = W1[gc][:, hidx]
        b1[gc] = b1[gc][hidx]
        W2[gc] = W2[gc][hidx]

    idx_by_company = [np.nonzero(cid == gc)[0] for gc in range(C)]
    max_cnt = max((len(ix) for ix in idx_by_company), default=1)
    max_cnt = max(max_cnt, 1)
    if max_cnt <= 128:
        NTT = 1
        # exact capacity (even, >=16): layer-1/gelu/z-DMA all scale with TW
        TW = max(((max_cnt + 1) // 2) * 2, 16)
    else:
        NTT = (max_cnt + 127) // 128
        TW = 128
    CAP = NTT * TW

    nc = _get_compiled(TW, NTT, dtype_name)

    in_maps = []
    for core in range(NCORES):
        # zt[p, c, tt, k, t] = z[token, 128k+p]  (partition-major)
        zt = np.zeros((128, CPC, NTT, KC, TW), dtype=np_op)
        for ci in range(CPC):
            gc = core * CPC + ci
            ix = idx_by_company[gc]
            if len(ix) == 0:
                continue
            zpad = np.zeros((CAP, D), dtype=np_op)
            zpad[: len(ix)] = z[ix].astype(np_op)
            zt[:, ci] = zpad.reshape(NTT, TW, KC, 128).transpose(3, 0, 2, 1)
        # w1[c, p, g, k, hh] = W1[gc, 128k+p, 512g+hh], packed as bytes:
        # h-chunks 0-1 in fp16 (512B), h-chunks 2-3 in fp8e4m3 (256B).
        import ml_dtypes

        w1f = (
            W1[core * CPC:(core + 1) * CPC]
            .reshape(CPC, KC, 128, 2, H // 2)
            .transpose(0, 2, 3, 1, 4)
        )
        w1_hi = np.ascontiguousarray(w1f[..., : 2 * 128]).astype(np_op)
        w1_lo = np.ascontiguousarray(w1f[..., 2 * 128:]).astype(
            ml_dtypes.float8_e4m3fn
        )
        w1 = np.concatenate(
            [
                w1_hi.view(np.uint8).reshape(CPC, 128, 2, KC, 512),
                w1_lo.view(np.uint8).reshape(CPC, 128, 2, KC, 256),
            ],
            axis=-1,
        )
        # w2h[p, HC*c + j] = W2[gc, 128j+p, 0] (fp16)
        w2h = (
            W2[core * CPC:(core + 1) * CPC, :, 0]
            .reshape(CPC, HC, 128)
            .transpose(2, 0, 1)
            .reshape(128, CPC * HC)
            .astype(np_op)
        )
        # b1p[p, (c*2+g)*KC + j] = b1[gc, 512g+128j+p] (fp32)
        b1p = (
            b1[core * CPC:(core + 1) * CPC]
            .reshape(CPC, 2, KC, 128)
            .transpose(3, 0, 1, 2)
            .reshape(128, CPC * 2 * KC)
            .astype(np.float32)
        )
        wb = np.concatenate(
            [
                np.ascontiguousarray(w2h).view(np.uint8),
                np.ascontiguousarray(b1p).view(np.uint8),
            ],
            axis=1,
        )
        in_maps.append(
            {
                "zt": np.ascontiguousarray(zt),
                "w1": np.ascontiguousarray(w1),
                "wb": np.ascontiguousarray(wb),
            }
        )

    res = run_bass_kernel_spmd(nc, in_maps, list(range(NCORES)))

    out = np.zeros((B, O), dtype=np.float32)
    for core in range(NCORES):
        core_out = res.results[core]["out"].reshape(CPC, NTT * TW)
        for ci in range(CPC):
            gc = core * CPC + ci
            ix = idx_by_company[gc]
            if len(ix) == 0:
                continue
            out[ix, 0] = core_out[ci, : len(ix)] + b2[gc, 0]
    return out


# revision 40
# speedup vs baseline: 1.0267x; 1.0267x over previous
"""Trainium2 Bass kernel for CompanySpecificHeads (MoE-style routed MLP heads).

Semantics (matching the reference):
    out[b] = gelu(z[b] @ W1[cid[b]] + b1[cid[b]]) @ W2[cid[b]] + b2[cid[b]]

Strategy: expert-parallel across 8 NeuronCores. Companies are sharded
8-per-core; tokens are routed (gathered by company) to their company's core
on the host, padded to a fixed per-company capacity, and each core runs a
grouped GEMM -> gelu -> dot pipeline over its 8 companies.

v2 structure (per company c, h on partitions):
  Bias prefill (DVE): the b1 slice for each (c, g) group is broadcast-copied
      into the PSUM bank BEFORE layer-1 runs. Layer-1 matmuls then use
      start=False so the PE accumulates onto the bias. This works because
      the PSUM has_written bits stay set from the previous accumulation
      group on that bank (only start=True clears them); the warmup matmuls
      give every pp bank one full-width PE write before first use so the
      bits are set from the start. Removes the per-group bias selector
      matmul (N=384) from the PE critical path entirely.
  Layer 1 (PE): psum[h, t] += W1[c][d, h] * zT[c][d, t], fp16 moving
      operand, stationary mixed fp16/fp8 (see below), start=False.
  Gelu (ACT): one full-width activation per group, PSUM -> SBUF fp16.
  Layer 2 (PE, deferred): company c's 8 K=128 dot matmuls are emitted
      AFTER company c+1's layer-1, so the gelu of (c, g1) has a full
      company's worth of PE work to hide behind and layer-2 never stalls
      on the ACT engine.

Mixed-precision W1: per (g,k) the first 2 h-chunks (256 cols) are fp16 and
the last 2 h-chunks are fp8e4m3 (PE takes an fp8 stationary with an fp16
moving operand). Host permutes h per company so the smallest-|W2| chunks
take the fp8 slots (gelu is elementwise and layer-2 sums over h, so a
consistent h-permutation of W1/b1/W2 is exact). Packed as raw bytes:
[c][p][g][k][256*2B fp16 | 256*1B fp8], sliced by byte range + bitcast.

DMA: everything compute-gating rides the sync (SP HWDGE) ring in FIFO
need-order: zt[0:2], then per-company w1 with token slices interleaved
~1 company ahead. Per-transfer completion (the 16-SDMA-engine sem) trails
the byte stream by ~2-3us (write-receipt + slow-lane spread), so sizing
balances two effects: bigger transfers sustain ~420GB/s (vs ~330 for a
fine mix) but their sems fire late. c0/c1 are g-halved (compute starts
earlier at the head), c2..c6 are whole-company, c7's g1 is k-halved and
computed k-outer so only 4 matmuls wait on the stream's final sem. Only
the 48KB wb tile uses the scalar (ACT HWDGE) ring: that ring crawls
whenever the sync stream is active (same 16 SDMA engines). Host does the
unshard/scatter back to [B, 1] and adds b2 (exact, fp32).

PE warmup: the HAM clock gate holds an idle PE at 1.2GHz and takes ~3.4us
of sustained activity to un-throttle to 2.4GHz. A handful of warmup
matmuls on scratch data (memset on the otherwise-idle vector engine)
bridge the framework preamble to the first weight arrival and double as
the has_written coverage for the PSUM banks.
"""

import numpy as np

B, C, D, H = 4096, 64, 512, 1024
NCORES = 8
CPC = C // NCORES  # companies per core
KC = D // 128      # contraction chunks of 128
HC = H // 128      # h chunks of 128

_COMPILED = {}


def _build(TW, NTT, dtype_name):
    """Build the Bass/Tile program for per-company token capacity NTT*TW."""
    import concourse.bass as bass
    import concourse.bacc as bacc
    import concourse.mybir as mybir
    from concourse.tile import TileContext
    from contextlib import ExitStack

    f32 = mybir.dt.float32
    dt_op = getattr(mybir.dt, dtype_name)
    f8 = mybir.dt.float8e4
    u8 = mybir.dt.uint8

    # Packed W1 bytes per (g,k): 2 h-chunks fp16 + 2 h-chunks fp8e4m3.
    W1B = 2 * 128 * 2 + 2 * 128  # = 768 bytes per (g,k)
    # wb layout per partition: [0:2*CPC*HC] w2 fp16, then b1 fp32.
    W2BYTES = CPC * HC * 2          # 128B
    B1BYTES = CPC * 2 * KC * 4      # 256B
    WBW = W2BYTES + B1BYTES

    nc = bacc.Bacc(None, target_bir_lowering=False)

    zt_d = nc.dram_tensor("zt", [128, CPC, NTT, KC, TW], dt_op, kind="ExternalInput")
    w1_d = nc.dram_tensor(
        "w1", [CPC, 128, 2, KC, W1B], u8, kind="ExternalInput"
    )
    wb_d = nc.dram_tensor("wb", [128, WBW], u8, kind="ExternalInput")
    out_d = nc.dram_tensor("out", [1, CPC * NTT * TW], f32, kind="ExternalOutput")

    gelu = mybir.ActivationFunctionType.Gelu

    with TileContext(nc) as tc, ExitStack() as ctx:
        const = ctx.enter_context(tc.tile_pool(name="const", bufs=1))

        # PE warmup scratch: memset on the otherwise-idle vector engine so
        # the warmup matmuls have no dependency on the DMA queues.
        wsc = const.tile([128, KC * TW], dt_op)
        nc.vector.memset(wsc[:], 0.0)

        # The scalar (ACT) HWDGE ring carries ONLY the 48KB wb tile: the
        # scalar ring crawls (~30-100GB/s) whenever the sync stream is
        # active (same 16 SDMA engines, packet round-robin strongly
        # favors the sync queue), so anything compute-gating must ride
        # the sync ring in FIFO need-order.
        wbt = const.tile([128, WBW], u8)
        nc.scalar.dma_start(out=wbt[:], in_=wb_d[:])
        w2t = wbt[:, 0:W2BYTES].bitcast(dt_op)              # [128, CPC*HC]
        b1sb = wbt[:, W2BYTES:WBW].bitcast(f32)             # [128, CPC*2*KC]
        zall = const.tile([128, CPC, NTT, KC, TW], dt_op)
        nc.sync.dma_start(out=zall[:, 0:2], in_=zt_d[:, 0:2])

        # Staged per-company outputs; two sync-ring stores at the end.
        oall = const.tile([1, CPC * NTT * TW], f32)

        # w1 on the sync ring. Transfer sizing balances two measured
        # effects: (a) per-transfer overhead + the 16-engine completion
        # spread reward FEWER, BIGGER transfers (whole-company 786KB
        # pieces sustain ~420GB/s vs ~360 for a finer mix); (b) a company
        # split in g-halves lets its layer-1 g0 start ~1us earlier.
        # So: c0/c1 (head, compute-gating) and c7 (its g1 completion sits
        # on the critical tail) are halved; c2..c6 are whole-company.
        # Token slices zt[2:] are interleaved into the stream ~1 company
        # ahead of need.
        w1p = ctx.enter_context(tc.tile_pool(name="w1p", bufs=1))
        w1ts = []
        for c in range(CPC):
            w1t = w1p.tile([128, 2, KC, W1B], u8, name=f"w1_{c}")
            if c in (0, 1):
                # head: g-halves so compute starts ~1us earlier (the
                # completion sem of a large transfer fires late)
                nc.sync.dma_start(out=w1t[:, 0], in_=w1_d[c, :, 0])
                nc.sync.dma_start(out=w1t[:, 1], in_=w1_d[c, :, 1])
            elif c == CPC - 1:
                # tail: g1 in k-halves computed k-outer, so only the last
                # 4 matmuls wait on the stream's final completion sem
                nc.sync.dma_start(out=w1t[:, 0], in_=w1_d[c, :, 0])
                nc.sync.dma_start(out=w1t[:, 1, 0:2], in_=w1_d[c, :, 1, 0:2])
                nc.sync.dma_start(out=w1t[:, 1, 2:4], in_=w1_d[c, :, 1, 2:4])
            else:
                nc.sync.dma_start(out=w1t[:], in_=w1_d[c])
            w1ts.append(w1t)
            if c == 1:
                nc.sync.dma_start(out=zall[:, 2:3], in_=zt_d[:, 2:3])
            elif c == 2:
                nc.sync.dma_start(out=zall[:, 3:4], in_=zt_d[:, 3:4])
            elif c == 3:
                nc.sync.dma_start(out=zall[:, 4:6], in_=zt_d[:, 4:6])
            elif c == 4:
                nc.sync.dma_start(out=zall[:, 6:8], in_=zt_d[:, 6:8])

        hp = ctx.enter_context(tc.tile_pool(name="hp", bufs=min(2 * CPC * NTT, 16)))
        pp = ctx.enter_context(tc.tile_pool(name="pp", bufs=5, space="PSUM"))
        opp = ctx.enter_context(tc.tile_pool(name="opp", bufs=2, space="PSUM"))
        wps = ctx.enter_context(tc.tile_pool(name="wps", bufs=1, space="PSUM"))

        # PE warmup matmuls, two jobs: (1) keep the PE's HAM busy-window
        # filling continuously from the end of the framework preamble to
        # the first weight arrival -- any idle gap >= one 4096-cycle
        # window resets the un-throttle progress and the whole stream
        # runs at 1.2GHz instead of 2.4GHz; (2) the first pass through
        # the pp pool gives every pp PSUM bank one full-width PE write so
        # has_written is set before the first bias prefill + start=False
        # accumulation.
        for _ in range(11):
            wp = pp.tile([128, KC * TW], f32, name="ps", tag="ps")
            nc.tensor.matmul(wp[:], wsc[:, :128], wsc[:], start=True, stop=True)

        # Dependency-free fillers on a dedicated bank: bridge the
        # DMA-paced holes at the head of the stream (c0/c1) so the PE
        # stays continuously busy until the weight stream runs ahead.
        wfil = wps.tile([128, 256], f32)

        def filler(n):
            for _ in range(n):
                nc.tensor.matmul(wfil[:], wsc[:, :128], wsc[:, :256],
                                 start=True, stop=True)

        def emit_l2(c, tt, hts):
            osum = opp.tile([1, TW], f32)
            for g in range(2):
                for j in range(KC):
                    jj = KC * g + j
                    nc.tensor.matmul(
                        osum[:],
                        w2t[:, HC * c + jj:HC * c + jj + 1],
                        hts[g][:, j * TW:(j + 1) * TW],
                        start=(jj == 0),
                        stop=(jj == HC - 1),
                    )
            off = (c * NTT + tt) * TW
            nc.vector.tensor_copy(oall[:, off:off + TW], osum[:])

        prev = None
        for c in range(CPC):
            w1t = w1ts[c]
            for tt in range(NTT):
                hts = []
                for g in range(2):
                    ps = pp.tile([128, KC * TW], f32, name="ps", tag="ps")
                    # bias prefill on DVE: ps[128j+m, t] = b1[c][512g+128j+m]
                    bb = b1sb[:, (c * 2 + g) * KC:(c * 2 + g) * KC + KC]
                    nc.vector.tensor_copy(
                        ps.rearrange("p (j t) -> p j t", j=KC),
                        bb.unsqueeze(-1).to_broadcast((128, KC, TW)),
                    )
                    tail = c == CPC - 1 and tt == NTT - 1
                    kj = [(k, j) for k in range(KC) for j in range(KC)] \
                        if (tail and g == 1) else \
                        [(k, j) for j in range(KC) for k in range(KC)]
                    for k, j in kj:
                        if True:
                            if j < 2:
                                lhsT = w1t[:, g, k, 256 * j:256 * (j + 1)].bitcast(
                                    dt_op
                                )
                            else:
                                lhsT = w1t[
                                    :, g, k, 512 + 128 * (j - 2):512 + 128 * (j - 1)
                                ].bitcast(f8)
                            nc.tensor.matmul(
                                ps[:, j * TW:(j + 1) * TW],
                                lhsT,
                                zall[:, c, tt, k, :],
                                start=False,
                                stop=(k == KC - 1),
                                skip_group_check=True,
                            )
                    ht = hp.tile([128, KC * TW], dt_op)
                    if c == CPC - 1 and tt == NTT - 1 and g == 1:
                        # tail: halve the last gelu so layer-2's second
                        # half starts on the first piece
                        half = KC * TW // 2
                        nc.scalar.activation(ht[:, :half], ps[:, :half], gelu)
                        nc.scalar.activation(ht[:, half:], ps[:, half:], gelu)
                    else:
                        nc.scalar.activation(ht[:], ps[:], gelu)
                    hts.append(ht)
                    if c == 0 and tt == 0 and g == 0:
                        filler(10)
                    if tail and g == 0 and prev is not None:
                        # tail: drain the previous company's layer-2
                        # between the last company's two groups, so the
                        # final gelus aren't scheduled behind the
                        # DMA-late L1(g1)
                        emit_l2(*prev)
                        prev = None
                if prev is not None:
                    emit_l2(*prev)
                prev = (c, tt, hts)
            if c == 0:
                filler(10)
            elif c == 1:
                filler(8)
            elif c == 2:
                filler(6)
        emit_l2(*prev)

        # Stores on the sync ring: companies 0-6 fire while company 7's
        # tail drains; the final store is tiny.
        osplit = (CPC - 1) * NTT * TW
        nc.sync.dma_start(out=out_d[:, :osplit], in_=oall[:, :osplit])
        nc.sync.dma_start(out=out_d[:, osplit:], in_=oall[:, osplit:])

    nc.finalize()
    return nc


def _get_compiled(TW, NTT, dtype_name):
    key = (TW, NTT, dtype_name)
    if key not in _COMPILED:
        _COMPILED[key] = _build(TW, NTT, dtype_name)
    return _COMPILED[key]


def kernel(z, company_id, W1, b1, W2, b2):
    from concourse.bass_utils import run_bass_kernel_spmd

    z = np.asarray(z, dtype=np.float32)
    cid = np.asarray(company_id).astype(np.int64).ravel()
    W1 = np.asarray(W1, dtype=np.float32)
    b1 = np.asarray(b1, dtype=np.float32)
    W2 = np.asarray(W2, dtype=np.float32)
    b2 = np.asarray(b2, dtype=np.float32)
    O = W2.shape[2]

    np_op = np.float16
    dtype_name = "float16"

    # Per-company h-chunk permutation: the packed layout quantizes chunks
    # 2,3 of each half to fp8, so route the 4 chunks with the SMALLEST
    # |W2| norms into those slots (gelu is elementwise and layer-2 sums
    # over h, so a consistent permutation of W1/b1/W2 along h is exact).
    W1 = W1.copy()
    b1 = b1.copy()
    W2 = W2.copy()
    for gc in range(C):
        norms = (W2[gc, :, 0].reshape(8, 128) ** 2).sum(1)
        order = np.argsort(-norms)
        slot_assign = np.empty(8, dtype=int)
        slot_assign[[0, 1, 4, 5]] = order[:4]
        slot_assign[[2, 3, 6, 7]] = order[4:]
        hidx = (slot_assign[:, None] * 128 + np.arange(128)).ravel()
        W1[gc] = W1[gc][:, hidx]
        b1[gc] = b1[gc][hidx]
        W2[gc] = W2[gc][hidx]

    idx_by_company = [np.nonzero(cid == gc)[0] for gc in range(C)]
    max_cnt = max((len(ix) for ix in idx_by_company), default=1)
    max_cnt = max(max_cnt, 1)
    if max_cnt <= 128:
        NTT = 1
        # exact capacity (even, >=16): layer-1/gelu/z-DMA all scale with TW
        TW = max(((max_cnt + 1) // 2) * 2, 16)
    else:
        NTT = (max_cnt + 127) // 128
        TW = 128
    CAP = NTT * TW

    nc = _get_compiled(TW, NTT, dtype_name)

    in_maps = []
    for core in range(NCORES):
        # zt[p, c, tt, k, t] = z[token, 128k+p]  (partition-major)
        zt = np.zeros((128, CPC, NTT, KC, TW), dtype=np_op)
        for ci in range(CPC):
            gc = core * CPC + ci
            ix = idx_by_company[gc]
            if len(ix) == 0:
                continue
            zpad = np.zeros((CAP, D), dtype=np_op)
            zpad[: len(ix)] = z[ix].astype(np_op)
            zt[:, ci] = zpad.reshape(NTT, TW, KC, 128).transpose(3, 0, 2, 1)
        # w1[c, p, g, k, hh] = W1[gc, 128k+p, 512g+hh], packed as bytes:
        # h-chunks 0-1 in fp16 (512B), h-chunks 2-3 in fp8e4m3 (256B).
        import ml_dtypes

        w1f = (
            W1[core * CPC:(core + 1) * CPC]
            .reshape(CPC, KC, 128, 2, H // 2)
            .transpose(0, 2, 3, 1, 4)
        )
        w1_hi = np.ascontiguousarray(w1f[..., : 2 * 128]).astype(np_op)
        w1_lo = np.ascontiguousarray(w1f[..., 2 * 128:]).astype(
            ml_dtypes.float8_e4m3fn
        )
        w1 = np.concatenate(
            [
                w1_hi.view(np.uint8).reshape(CPC, 128, 2, KC, 512),
                w1_lo.view(np.uint8).reshape(CPC, 128, 2, KC, 256),
            ],
            axis=-1,
        )
        # w2h[p, HC*c + j] = W2[gc, 128j+p, 0] (fp16)
        w2h = (
            W2[core * CPC:(core + 1) * CPC, :, 0]
            .reshape(CPC, HC, 128)
            .transpose(2, 0, 1)
            .reshape(128, CPC * HC)
            .astype(np_op)
        )
        # b1p[p, (c*2+g)*KC + j] = b1[gc, 512g+128j+p] (fp32)
        b1p = (
            b1[core * CPC:(core + 1) * CPC]
            .reshape(CPC, 2, KC, 128)
            .transpose(3, 0, 1, 2)
            .reshape(128, CPC * 2 * KC)
            .astype(np.float32)
        )
        wb = np.concatenate(
            [
                np.ascontiguousarray(w2h).view(np.uint8),
                np.ascontiguousarray(b1p).view(np.uint8),
            ],
            axis=1,
        )
        in_maps.append(
            {
                "zt": np.ascontiguousarray(zt),
                "w1": np.ascontiguousarray(w1),
                "wb": np.ascontiguousarray(wb),
            }
        )

    res = run_bass_kernel_spmd(nc, in_maps, list(range(NCORES)))

    out = np.zeros((B, O), dtype=np.float32)
    for core in range(NCORES):
        core_out = res.results[core]["out"].reshape(CPC, NTT * TW)
        for ci in range(CPC):
            gc = core * CPC + ci
            ix = idx_by_company[gc]
            if len(ix) == 0:
                continue
            out[ix, 0] = core_out[ci, : len(ix)] + b2[gc, 0]
    return out


# revision 41
# speedup vs baseline: 1.0304x; 1.0036x over previous
"""Trainium2 Bass kernel for CompanySpecificHeads (MoE-style routed MLP heads).

Semantics (matching the reference):
    out[b] = gelu(z[b] @ W1[cid[b]] + b1[cid[b]]) @ W2[cid[b]] + b2[cid[b]]

Strategy: expert-parallel across 8 NeuronCores. Companies are sharded
8-per-core; tokens are routed (gathered by company) to their company's core
on the host, padded to a fixed per-company capacity, and each core runs a
grouped GEMM -> gelu -> dot pipeline over its 8 companies.

v2 structure (per company c, h on partitions):
  Bias prefill (DVE): the b1 slice for each (c, g) group is broadcast-copied
      into the PSUM bank BEFORE layer-1 runs. Layer-1 matmuls then use
      start=False so the PE accumulates onto the bias. This works because
      the PSUM has_written bits stay set from the previous accumulation
      group on that bank (only start=True clears them); the warmup matmuls
      give every pp bank one full-width PE write before first use so the
      bits are set from the start. Removes the per-group bias selector
      matmul (N=384) from the PE critical path entirely.
  Layer 1 (PE): psum[h, t] += W1[c][d, h] * zT[c][d, t], fp16 moving
      operand, stationary mixed fp16/fp8 (see below), start=False.
  Gelu (ACT): one full-width activation per group, PSUM -> SBUF fp16.
  Layer 2 (PE, deferred): company c's 8 K=128 dot matmuls are emitted
      AFTER company c+1's layer-1, so the gelu of (c, g1) has a full
      company's worth of PE work to hide behind and layer-2 never stalls
      on the ACT engine.

Mixed-precision W1: per (g,k) the first 2 h-chunks (256 cols) are fp16 and
the last 2 h-chunks are fp8e4m3 (PE takes an fp8 stationary with an fp16
moving operand). Host permutes h per company so the smallest-|W2| chunks
take the fp8 slots (gelu is elementwise and layer-2 sums over h, so a
consistent h-permutation of W1/b1/W2 is exact). Packed as raw bytes:
[c][p][g][k][256*2B fp16 | 256*1B fp8], sliced by byte range + bitcast.

DMA: everything compute-gating rides the sync (SP HWDGE) ring in FIFO
need-order: zt[0:2], then per-company w1 with token slices interleaved
~1 company ahead. Per-transfer completion (the 16-SDMA-engine sem) trails
the byte stream by ~2-3us (write-receipt + slow-lane spread), so sizing
balances two effects: bigger transfers sustain ~420GB/s (vs ~330 for a
fine mix) but their sems fire late. c0/c1 are g-halved (compute starts
earlier at the head), c2..c6 are whole-company, c7's g1 is k-halved and
computed k-outer so only 4 matmuls wait on the stream's final sem. Only
the 48KB wb tile uses the scalar (ACT HWDGE) ring: that ring crawls
whenever the sync stream is active (same 16 SDMA engines). Host does the
unshard/scatter back to [B, 1] and adds b2 (exact, fp32).

PE warmup: the HAM clock gate holds an idle PE at 1.2GHz and takes ~3.4us
of sustained activity to un-throttle to 2.4GHz. A handful of warmup
matmuls on scratch data (memset on the otherwise-idle vector engine)
bridge the framework preamble to the first weight arrival and double as
the has_written coverage for the PSUM banks.
"""

import numpy as np

B, C, D, H = 4096, 64, 512, 1024
NCORES = 8
CPC = C // NCORES  # companies per core
KC = D // 128      # contraction chunks of 128
HC = H // 128      # h chunks of 128

_COMPILED = {}


def _build(TW, NTT, dtype_name):
    """Build the Bass/Tile program for per-company token capacity NTT*TW."""
    import concourse.bass as bass
    import concourse.bacc as bacc
    import concourse.mybir as mybir
    from concourse.tile import TileContext
    from contextlib import ExitStack

    f32 = mybir.dt.float32
    dt_op = getattr(mybir.dt, dtype_name)
    f8 = mybir.dt.float8e4
    u8 = mybir.dt.uint8

    # Packed W1 bytes per (g,k): 2 h-chunks fp16 + 2 h-chunks fp8e4m3.
    W1B = 2 * 128 * 2 + 2 * 128  # = 768 bytes per (g,k)
    # wb layout per partition: [0:2*CPC*HC] w2 fp16, then b1 fp32.
    W2BYTES = CPC * HC * 2          # 128B
    B1BYTES = CPC * 2 * KC * 4      # 256B
    WBW = W2BYTES + B1BYTES

    nc = bacc.Bacc(None, target_bir_lowering=False)

    zt_d = nc.dram_tensor("zt", [128, CPC, NTT, KC, TW], dt_op, kind="ExternalInput")
    w1_d = nc.dram_tensor(
        "w1", [CPC, 128, 2, KC, W1B], u8, kind="ExternalInput"
    )
    wb_d = nc.dram_tensor("wb", [128, WBW], u8, kind="ExternalInput")
    out_d = nc.dram_tensor("out", [1, CPC * NTT * TW], f32, kind="ExternalOutput")

    gelu = mybir.ActivationFunctionType.Gelu

    with TileContext(nc) as tc, ExitStack() as ctx:
        const = ctx.enter_context(tc.tile_pool(name="const", bufs=1))

        # PE warmup scratch: memset on the otherwise-idle vector engine so
        # the warmup matmuls have no dependency on the DMA queues.
        wsc = const.tile([128, KC * TW], dt_op)
        nc.vector.memset(wsc[:], 0.0)

        # Everything rides the sync ring in FIFO need-order -- even the
        # 48KB wb tile: any scalar-ring transfer steals per-packet SDMA
        # slots from the sync stream head (same 16 engines), and the
        # head completions gate both compute start and HAM warm-up.
        wbt = const.tile([128, WBW], u8)
        nc.sync.dma_start(out=wbt[:], in_=wb_d[:])
        w2t = wbt[:, 0:W2BYTES].bitcast(dt_op)              # [128, CPC*HC]
        b1sb = wbt[:, W2BYTES:WBW].bitcast(f32)             # [128, CPC*2*KC]
        zall = const.tile([128, CPC, NTT, KC, TW], dt_op)
        nc.sync.dma_start(out=zall[:, 0:2], in_=zt_d[:, 0:2])

        # Staged per-company outputs; two sync-ring stores at the end.
        oall = const.tile([1, CPC * NTT * TW], f32)

        # w1 on the sync ring. Transfer sizing balances two measured
        # effects: (a) per-transfer overhead + the 16-engine completion
        # spread reward FEWER, BIGGER transfers (whole-company 786KB
        # pieces sustain ~420GB/s vs ~360 for a finer mix); (b) a company
        # split in g-halves lets its layer-1 g0 start ~1us earlier.
        # So: c0/c1 (head, compute-gating) and c7 (its g1 completion sits
        # on the critical tail) are halved; c2..c6 are whole-company.
        # Token slices zt[2:] are interleaved into the stream ~1 company
        # ahead of need.
        w1p = ctx.enter_context(tc.tile_pool(name="w1p", bufs=1))
        w1ts = []
        for c in range(CPC):
            w1t = w1p.tile([128, 2, KC, W1B], u8, name=f"w1_{c}")
            if c in (0, 1):
                # head: g-halves so compute starts ~1us earlier (the
                # completion sem of a large transfer fires late)
                nc.sync.dma_start(out=w1t[:, 0], in_=w1_d[c, :, 0])
                nc.sync.dma_start(out=w1t[:, 1], in_=w1_d[c, :, 1])
            elif c == CPC - 1:
                # tail: g1 in k-halves computed k-outer, so only the last
                # 4 matmuls wait on the stream's final completion sem
                nc.sync.dma_start(out=w1t[:, 0], in_=w1_d[c, :, 0])
                nc.sync.dma_start(out=w1t[:, 1, 0:2], in_=w1_d[c, :, 1, 0:2])
                nc.sync.dma_start(out=w1t[:, 1, 2:4], in_=w1_d[c, :, 1, 2:4])
            else:
                nc.sync.dma_start(out=w1t[:], in_=w1_d[c])
            w1ts.append(w1t)
            if c == 1:
                nc.sync.dma_start(out=zall[:, 2:3], in_=zt_d[:, 2:3])
            elif c == 2:
                nc.sync.dma_start(out=zall[:, 3:4], in_=zt_d[:, 3:4])
            elif c == 3:
                nc.sync.dma_start(out=zall[:, 4:6], in_=zt_d[:, 4:6])
            elif c == 4:
                nc.sync.dma_start(out=zall[:, 6:8], in_=zt_d[:, 6:8])

        hp = ctx.enter_context(tc.tile_pool(name="hp", bufs=min(2 * CPC * NTT, 16)))
        pp = ctx.enter_context(tc.tile_pool(name="pp", bufs=5, space="PSUM"))
        opp = ctx.enter_context(tc.tile_pool(name="opp", bufs=2, space="PSUM"))
        wps = ctx.enter_context(tc.tile_pool(name="wps", bufs=1, space="PSUM"))

        # PE warmup matmuls, two jobs: (1) keep the PE's HAM busy-window
        # filling continuously from the end of the framework preamble to
        # the first weight arrival -- any idle gap >= one 4096-cycle
        # window resets the un-throttle progress and the whole stream
        # runs at 1.2GHz instead of 2.4GHz; (2) the first pass through
        # the pp pool gives every pp PSUM bank one full-width PE write so
        # has_written is set before the first bias prefill + start=False
        # accumulation.
        for _ in range(11):
            wp = pp.tile([128, KC * TW], f32, name="ps", tag="ps")
            nc.tensor.matmul(wp[:], wsc[:, :128], wsc[:], start=True, stop=True)

        # Dependency-free fillers on a dedicated bank: bridge the
        # DMA-paced holes at the head of the stream (c0/c1) so the PE
        # stays continuously busy until the weight stream runs ahead.
        wfil = wps.tile([128, 256], f32)

        def filler(n):
            for _ in range(n):
                nc.tensor.matmul(wfil[:], wsc[:, :128], wsc[:, :256],
                                 start=True, stop=True)

        def emit_l2(c, tt, hts):
            osum = opp.tile([1, TW], f32)
            for g in range(2):
                for j in range(KC):
                    jj = KC * g + j
                    nc.tensor.matmul(
                        osum[:],
                        w2t[:, HC * c + jj:HC * c + jj + 1],
                        hts[g][:, j * TW:(j + 1) * TW],
                        start=(jj == 0),
                        stop=(jj == HC - 1),
                    )
            off = (c * NTT + tt) * TW
            nc.vector.tensor_copy(oall[:, off:off + TW], osum[:])

        prev = None
        for c in range(CPC):
            w1t = w1ts[c]
            for tt in range(NTT):
                hts = []
                for g in range(2):
                    ps = pp.tile([128, KC * TW], f32, name="ps", tag="ps")
                    # bias prefill on DVE: ps[128j+m, t] = b1[c][512g+128j+m]
                    bb = b1sb[:, (c * 2 + g) * KC:(c * 2 + g) * KC + KC]
                    nc.vector.tensor_copy(
                        ps.rearrange("p (j t) -> p j t", j=KC),
                        bb.unsqueeze(-1).to_broadcast((128, KC, TW)),
                    )
                    tail = c == CPC - 1 and tt == NTT - 1
                    kj = [(k, j) for k in range(KC) for j in range(KC)] \
                        if (tail and g == 1) else \
                        [(k, j) for j in range(KC) for k in range(KC)]
                    for k, j in kj:
                        if True:
                            if j < 2:
                                lhsT = w1t[:, g, k, 256 * j:256 * (j + 1)].bitcast(
                                    dt_op
                                )
                            else:
                                lhsT = w1t[
                                    :, g, k, 512 + 128 * (j - 2):512 + 128 * (j - 1)
                                ].bitcast(f8)
                            nc.tensor.matmul(
                                ps[:, j * TW:(j + 1) * TW],
                                lhsT,
                                zall[:, c, tt, k, :],
                                start=False,
                                stop=(k == KC - 1),
                                skip_group_check=True,
                            )
                    ht = hp.tile([128, KC * TW], dt_op)
                    if c == CPC - 1 and tt == NTT - 1 and g == 1:
                        # tail: halve the last gelu so layer-2's second
                        # half starts on the first piece
                        half = KC * TW // 2
                        nc.scalar.activation(ht[:, :half], ps[:, :half], gelu)
                        nc.scalar.activation(ht[:, half:], ps[:, half:], gelu)
                    else:
                        nc.scalar.activation(ht[:], ps[:], gelu)
                    hts.append(ht)
                    if c == 0 and tt == 0 and g == 0:
                        filler(10)
                    if tail and g == 0 and prev is not None:
                        # tail: drain the previous company's layer-2
                        # between the last company's two groups, so the
                        # final gelus aren't scheduled behind the
                        # DMA-late L1(g1)
                        emit_l2(*prev)
                        prev = None
                if prev is not None:
                    emit_l2(*prev)
                prev = (c, tt, hts)
            if c == 0:
                filler(10)
            elif c == 1:
                filler(8)
            elif c == 2:
                filler(6)
        emit_l2(*prev)

        # Stores on the sync ring: companies 0-6 fire while company 7's
        # tail drains; the final store is tiny.
        osplit = (CPC - 1) * NTT * TW
        nc.sync.dma_start(out=out_d[:, :osplit], in_=oall[:, :osplit])
        nc.sync.dma_start(out=out_d[:, osplit:], in_=oall[:, osplit:])

    nc.finalize()
    return nc


def _get_compiled(TW, NTT, dtype_name):
    key = (TW, NTT, dtype_name)
    if key not in _COMPILED:
        _COMPILED[key] = _build(TW, NTT, dtype_name)
    return _COMPILED[key]


def kernel(z, company_id, W1, b1, W2, b2):
    from concourse.bass_utils import run_bass_kernel_spmd

    z = np.asarray(z, dtype=np.float32)
    cid = np.asarray(company_id).astype(np.int64).ravel()
    W1 = np.asarray(W1, dtype=np.float32)
    b1 = np.asarray(b1, dtype=np.float32)
    W2 = np.asarray(W2, dtype=np.float32)
    b2 = np.asarray(b2, dtype=np.float32)
    O = W2.shape[2]

    np_op = np.float16
    dtype_name = "float16"

    # Per-company h-chunk permutation: the packed layout quantizes chunks
    # 2,3 of each half to fp8, so route the 4 chunks with the SMALLEST
    # |W2| norms into those slots (gelu is elementwise and layer-2 sums
    # over h, so a consistent permutation of W1/b1/W2 along h is exact).
    W1 = W1.copy()
    b1 = b1.copy()
    W2 = W2.copy()
    for gc in range(C):
        norms = (W2[gc, :, 0].reshape(8, 128) ** 2).sum(1)
        order = np.argsort(-norms)
        slot_assign = np.empty(8, dtype=int)
        slot_assign[[0, 1, 4, 5]] = order[:4]
        slot_assign[[2, 3, 6, 7]] = order[4:]
        hidx = (slot_assign[:, None] * 128 + np.arange(128)).ravel()
        W1[gc] = W1[gc][:, hidx]
        b1[gc] = b1[gc][hidx]
        W2[gc] = W2[gc][hidx]

    idx_by_company = [np.nonzero(cid == gc)[0] for gc in range(C)]
    max_cnt = max((len(ix) for ix in idx_by_company), default=1)
    max_cnt = max(max_cnt, 1)
    if max_cnt <= 128:
        NTT = 1
        # exact capacity (even, >=16): layer-1/gelu/z-DMA all scale with TW
        TW = max(((max_cnt + 1) // 2) * 2, 16)
    else:
        NTT = (max_cnt + 127) // 128
        TW = 128
    CAP = NTT * TW

    nc = _get_compiled(TW, NTT, dtype_name)

    in_maps = []
    for core in range(NCORES):
        # zt[p, c, tt, k, t] = z[token, 128k+p]  (partition-major)
        zt = np.zeros((128, CPC, NTT, KC, TW), dtype=np_op)
        for ci in range(CPC):
            gc = core * CPC + ci
            ix = idx_by_company[gc]
            if len(ix) == 0:
                continue
            zpad = np.zeros((CAP, D), dtype=np_op)
            zpad[: len(ix)] = z[ix].astype(np_op)
            zt[:, ci] = zpad.reshape(NTT, TW, KC, 128).transpose(3, 0, 2, 1)
        # w1[c, p, g, k, hh] = W1[gc, 128k+p, 512g+hh], packed as bytes:
        # h-chunks 0-1 in fp16 (512B), h-chunks 2-3 in fp8e4m3 (256B).
        import ml_dtypes

        w1f = (
            W1[core * CPC:(core + 1) * CPC]
            .reshape(CPC, KC, 128, 2, H // 2)
            .transpose(0, 2, 3, 1, 4)
        )
        w1_hi = np.ascontiguousarray(w1f[..., : 2 * 128]).astype(np_op)
        w1_lo = np.ascontiguousarray(w1f[..., 2 * 128:]).astype(
            ml_dtypes.float8_e4m3fn
        )
        w1 = np.concatenate(
            [
                w1_hi.view(np.uint8).reshape(CPC, 128, 2, KC, 512),
                w1_lo.view(np.uint8).reshape(CPC, 128, 2, KC, 256),
            ],
            axis=-1,
        )
        # w2h[p, HC*c + j] = W2[gc, 128j+p, 0] (fp16)
        w2h = (
            W2[core * CPC:(core + 1) * CPC, :, 0]
            .reshape(CPC, HC, 128)
            .transpose(2, 0, 1)
            .reshape(128, CPC * HC)
            .astype(np_op)
        )
        # b1p[p, (c*2+g)*KC + j] = b1[gc, 512g+128j+p] (fp32)
        b1p = (
            b1[core * CPC:(core + 1) * CPC]
            .reshape(CPC, 2, KC, 128)
            .transpose(3, 0, 1, 2)
            .reshape(128, CPC * 2 * KC)
            .astype(np.float32)
        )
        wb = np.concatenate(
            [
                np.ascontiguousarray(w2h).view(np.uint8),
                np.ascontiguousarray(b1p).view(np.uint8),
            ],
            axis=1,
        )
        in_maps.append(
            {
                "zt": np.ascontiguousarray(zt),
                "w1": np.ascontiguousarray(w1),
                "wb": np.ascontiguousarray(wb),
            }
        )

    res = run_bass_kernel_spmd(nc, in_maps, list(range(NCORES)))

    out = np.zeros((B, O), dtype=np.float32)
    for core in range(NCORES):
        core_out = res.results[core]["out"].reshape(CPC, NTT * TW)
        for ci in range(CPC):
            gc = core * CPC + ci
            ix = idx_by_company[gc]
            if len(ix) == 0:
                continue
            out[ix, 0] = core_out[ci, : len(ix)] + b2[gc, 0]
    return out
